# revision 42
# baseline (speedup 1.0000x reference)
"""Causal self-attention TRN2 kernel.

Problem: B=4, T=2048, C=1024, H=16 heads, Dh=64, fp32 I/O.

Sharding: 8 cores = 4 batches x 2 head-groups (8 heads each). Each core
computes QKV projection for its head-group, causal attention, and a partial
output projection; the host sums the two partials per batch and adds b_out.

Per-core layout (all matmul operands fp16; PSUM accumulation fp32):
  - xT [C, T] fp16 (host-transposed), Wq/Wk/Wv [C, 512] fp16, Wo [512, C] fp16
  - QT/KT [512, T] fp16 stored as 4 head-pair tiles [128, T] (partition = 2x64 dh)
  - V [T, 520] fp16 stored as 16 t-block tiles [128, 8, 65]: per head 64 V cols
    + a ones column (the AV matmul's ones column accumulates softmax row-sums)
  - scores computed transposed: ST [keys 128, queries 512] = KTh_blk.T @ QTh_chunk
    so exp(ST) feeds the AV matmul directly (no P transpose needed)
  - AV: OT [65, 512] += Vaug[128, 65].T @ P[128, 512]; row 64 = softmax sums
  - normalize: one DVE copy evicts OT (freeing its PSUM bank), then
    reciprocal of the sum row, GPSIMD partition-broadcast, and a DVE multiply
    run from SBUF without ever gating the PE's matmul stream; the AV pair for
    each score strip is also emitted one kb-pair behind its exp so the
    in-order PE never waits on the ACT engine
  - out proj: yT [C, T] fp32 partial = Wo_hp.T @ OT_hp accumulated over head pairs

loop_iters>1 wraps the whole body in a tc.For_i hardware loop (timing only).
"""

import numpy as np

import concourse.bacc as bacc
import concourse.mybir as mybir
import concourse.tile as tile
from concourse import bass_utils

F32 = mybir.dt.float32
F16 = mybir.dt.float16
AF = mybir.ActivationFunctionType

B, T, C = 4, 2048, 1024
H, DH = 16, 64
HPC = 512          # head dims per core (8 heads x 64)
NHP = 4            # head pairs per core
NC_CHUNKS = C // 128   # 8 contraction chunks
NTS = T // 512     # 4 t-chunks of 512
NTB = T // 128     # 16 t-blocks of 128
SCALE = 1.0 / np.sqrt(DH)

_cache = {}


def _build(loop_iters=1, loop_phases=(1, 2, 3), ablate=None):
    key = ("nc", loop_iters, tuple(loop_phases), ablate)
    if key in _cache:
        return _cache[key]
    nc = bacc.Bacc(trn_type="TRN2", target_bir_lowering=False, debug=False)

    xt = nc.dram_tensor("xt", [C, T], F16, kind="ExternalInput").ap()
    wq = nc.dram_tensor("wq", [C, HPC], F16, kind="ExternalInput").ap()
    wk = nc.dram_tensor("wk", [C, HPC], F16, kind="ExternalInput").ap()
    wv = nc.dram_tensor("wv", [C, HPC], F16, kind="ExternalInput").ap()
    wo = nc.dram_tensor("wo", [HPC, C], F16, kind="ExternalInput").ap()
    bqk = nc.dram_tensor("bqk", [128, 2 * NHP], F32, kind="ExternalInput").ap()
    bv = nc.dram_tensor("bv", [128, HPC], F16, kind="ExternalInput").ap()
    masks = nc.dram_tensor("masks", [128, 1280], F16, kind="ExternalInput").ap()
    yt = nc.dram_tensor("yt", [C, T], F32, kind="ExternalOutput").ap()

    with tile.TileContext(nc) as tc:
        with (
            tc.tile_pool(name="wp", bufs=1) as wp,          # persistent weights/consts
            tc.tile_pool(name="big", bufs=1) as big,        # QT/KT/V/OT persistent
            tc.tile_pool(name="xs", bufs=1) as xs,          # streamed xT chunks
            tc.tile_pool(name="ev", bufs=3) as ev,          # small sbuf staging
            tc.tile_pool(name="ps", bufs=1, space="PSUM") as ps,
        ):
            # ---- persistent loads (outside the timing loop) ----
            wq_t, wk_t, wv_t = [], [], []
            for c in range(NC_CHUNKS):
                wqc = wp.tile([128, HPC], F16, name=f"wq{c}", tag=f"wq{c}")
                nc.sync.dma_start(wqc[:], wq[c * 128:(c + 1) * 128, :])
                wq_t.append(wqc)
                wkc = wp.tile([128, HPC], F16, name=f"wk{c}", tag=f"wk{c}")
                nc.sync.dma_start(wkc[:], wk[c * 128:(c + 1) * 128, :])
                wk_t.append(wkc)
                wvc = wp.tile([128, HPC], F16, name=f"wv{c}", tag=f"wv{c}")
                nc.sync.dma_start(wvc[:], wv[c * 128:(c + 1) * 128, :])
                wv_t.append(wvc)
            bqk_t = wp.tile([128, 2 * NHP], F32, name="bqk_t", tag="bqk")
            nc.sync.dma_start(bqk_t[:], bqk)
            bv_t = wp.tile([128, HPC], F16, name="bv_t", tag="bv")
            nc.sync.dma_start(bv_t[:], bv)
            mask_t = wp.tile([128, 1280], F16, name="mask_t", tag="mask")
            nc.sync.dma_start(mask_t[:], masks)
            wo_t = []
            for hp in range(NHP):
                woc = wp.tile([128, C], F16, name=f"wo{hp}", tag=f"wo{hp}")
                nc.sync.dma_start(woc[:], wo[hp * 128:(hp + 1) * 128, :])
                wo_t.append(woc)
            if ablate is not None:
                pconst = wp.tile([128, 1024], F16, name="pconst", tag="pconst")
                nc.vector.memset(pconst[:], 2.0 ** -11)
                bcdummy2 = wp.tile([64, 2048], F32, name="bcdummy2", tag="bcdummy")
                nc.vector.memset(bcdummy2[:], 1.0)

            qt_t = [big.tile([128, T], F16, name=f"qt{i}", tag=f"qt{i}") for i in range(NHP)]
            kt_t = [big.tile([128, T], F16, name=f"kt{i}", tag=f"kt{i}") for i in range(NHP)]
            v_t = [big.tile([128, 8, 65], F16, name=f"v{i}", tag=f"v{i}") for i in range(NTB)]
            ot_t = [big.tile([128, T], F16, name=f"ot{i}", tag=f"ot{i}") for i in range(NHP)]

            def body(phases=(1, 2, 3), real=False):
                if tuple(phases) == (1, 2, 3):
                    # interleaved: QK for head-pairs 1-3 emitted as quanta
                    # inside phase 2's kb-pair stream to fill PE stall bubbles
                    xrow = phase1_load()
                    phase1_qk(xrow, 0)
                    phase1_v(xrow)
                    queues = {hp: qk_quanta(xrow, hp) for hp in (1, 2, 3)}
                    phase2(real=real, qk_queues=queues)
                    phase3()
                    return
                if 1 in phases:
                    phase1()
                if 2 in phases:
                    phase2(real=real)
                if 3 in phases:
                    phase3()

            def phase1_load():
                xrow = []
                for c in range(NC_CHUNKS):
                    xr = xs.tile([128, T], F16, name=f"x_{c}", tag=f"x{c}")
                    nc.sync.dma_start(xr[:], xt[c * 128:(c + 1) * 128, :])
                    xrow.append(xr)
                return xrow

            def qk_quanta(xrow, hp):
                # list of zero-arg closures: 8 matmuls + 1 eviction per (ts, q/k)
                quanta = []
                for ts in range(NTS):
                    xc = [xr[:, ts * 512:(ts + 1) * 512] for xr in xrow]
                    for which in (0, 1):
                        w_t = wq_t if which == 0 else wk_t
                        dst = (qt_t if which == 0 else kt_t)[hp]
                        bias_col = hp if which == 0 else NHP + hp
                        p = ps.tile([128, 512], F32,
                                    name=f"pqk_{hp}_{ts}_{which}", tag="st", bufs=3)

                        def mk_mm(p=p, w_t=w_t, xc=xc, hp=hp, c=0):
                            return lambda: nc.tensor.matmul(
                                p[:], w_t[c][:, hp * 128:(hp + 1) * 128], xc[c],
                                start=(c == 0), stop=(c == NC_CHUNKS - 1))
                        for c in range(NC_CHUNKS):
                            quanta.append(mk_mm(c=c))

                        def mk_ev(p=p, dst=dst, ts=ts, bias_col=bias_col):
                            return lambda: nc.vector.tensor_scalar_add(
                                dst[:, ts * 512:(ts + 1) * 512], p[:],
                                bqk_t[:, bias_col:bias_col + 1])
                        quanta.append(mk_ev())
                return quanta

            def phase1_qk(xrow, hp):
                for q in qk_quanta(xrow, hp):
                    q()

            def phase1_v(xrow):
                for ts in range(NTS):
                    xc = [xr[:, ts * 512:(ts + 1) * 512] for xr in xrow]
                    for tb in range(4):
                        pv = ps.tile([128, 512], F32, name=f"pv_{ts}_{tb}", tag="st", bufs=3)
                        for c in range(NC_CHUNKS):
                            nc.tensor.matmul(
                                pv[:], xc[c][:, tb * 128:(tb + 1) * 128], wv_t[c][:],
                                start=(c == 0), stop=(c == NC_CHUNKS - 1),
                            )
                        vt = v_t[ts * 4 + tb]
                        nc.vector.tensor_add(
                            vt[:, :, 0:64],
                            pv[:].rearrange("p (h d) -> p h d", h=8),
                            bv_t[:].rearrange("p (h d) -> p h d", h=8),
                        )
                        nc.vector.memset(vt[:, :, 64:65], 1.0)

            def phase1():
                xrow = phase1_load()
                for hp in range(NHP):
                    phase1_qk(xrow, hp)
                phase1_v(xrow)

            def phase2(real=False, qk_queues=None):
                if not real and ablate in ("mm512", "mm512acc", "mmst", "mmav"):
                    # PE micro-benchmarks: 640 matmuls in phase-2's slot
                    for i in range(640):
                        st = ps.tile([128, 512], F32, name=f"mb_{i}", tag="st", bufs=3)
                        if ablate == "mm512":
                            nc.tensor.matmul(st[:], wq_t[0][:, 0:128], pconst[:, 0:512],
                                             start=True, stop=True)
                        elif ablate == "mm512acc":
                            nc.tensor.matmul(st[:], wq_t[0][:, 0:128], pconst[:, 0:512],
                                             start=(i % 8 == 0), stop=(i % 8 == 7))
                        elif ablate == "mmst":
                            nc.tensor.matmul(st[:], kt_t[0][0:64, 0:128],
                                             qt_t[0][0:64, 0:512], start=True, stop=True)
                        else:  # mmav
                            nc.tensor.matmul(st[0:65, :], v_t[i % 16][:, 0, :],
                                             pconst[:, 0:512], start=True, stop=True)
                    return
                # ---- phase 2: causal attention ----
                use_abl = (not real) and ablate in ("nonorm", "dumbc")

                def do_evict(ot, hp, off, j, h, state):
                    # prompt PSUM-freeing evict + recip slice; bcast/mul batched per head
                    oraw = ev.tile([65, 512], F32, name=f"or_{h}_{j}", tag="oraw", bufs=6)
                    nc.vector.tensor_copy(oraw[:], ot[:])
                    if use_abl and ablate == "nonorm":
                        nc.vector.tensor_copy(
                            ot_t[hp][off:off + 64, j * 512:(j + 1) * 512], oraw[0:64, :])
                        return
                    nc.vector.reciprocal(
                        state["recip"][:, j * 512:(j + 1) * 512], oraw[64:65, :])
                    state["oraw"][j] = oraw

                def flush_head(hp, off, h, state):
                    if use_abl and ablate == "nonorm":
                        return
                    if use_abl and ablate == "dumbc":
                        bc = bcdummy2
                    else:
                        bc = ev.tile([64, 2048], F32, name=f"bch_{h}", tag="bcs", bufs=2)
                        nc.gpsimd.partition_broadcast(bc[:], state["recip"][:])
                    for j, oraw in state["oraw"].items():
                        nc.vector.tensor_mul(
                            ot_t[hp][off:off + 64, j * 512:(j + 1) * 512],
                            oraw[0:64, :], bc[:, j * 512:(j + 1) * 512])

                norm_q = []
                pending = None  # deferred AV pair
                head_state = {}
                for h in range(8):
                    hp, off = h // 2, 64 * (h % 2)
                    if qk_queues:
                        # this head's QT/KT must be complete: drain stragglers
                        for hp2 in range(1, hp + 1):
                            while qk_queues.get(hp2):
                                qk_queues[hp2].pop(0)()
                    head_state[h] = {
                        "recip": ev.tile([1, 2048], F32, name=f"rch_{h}", tag="recip", bufs=2),
                        "oraw": {},
                    }
                    for j in range(NTS):
                        nkb = 4 * j + 4
                        ot = ps.tile([65, 512], F32, name=f"ot_{h}_{j}", tag="ot", bufs=2)
                        for m in range(nkb // 2):   # kb pairs
                            kb0, kb1 = 2 * m, 2 * m + 1
                            masked = kb1 >= 4 * j
                            if masked:
                                # diagonal blocks: restrict to the valid query
                                # tail q in [128r, 512) of this j-chunk
                                r0 = kb0 - 4 * j  # 0 or 2
                                w0, w1 = 512 - 128 * r0, 512 - 128 * (r0 + 1)
                                packoff = 0 if r0 == 0 else 896
                            else:
                                w0 = w1 = 512
                            q0 = j * 512
                            st = ps.tile([128, 1024], F32, name=f"st_{h}_{j}_{m}", tag="st", bufs=3)
                            for i, (kb, w) in enumerate(((kb0, w0), (kb1, w1))):
                                nc.tensor.matmul(
                                    st[:, i * w0:i * w0 + w],
                                    kt_t[hp][off:off + 64, kb * 128:(kb + 1) * 128],
                                    qt_t[hp][off:off + 64, q0 + 512 - w:q0 + 512],
                                    start=True, stop=True,
                                )
                            wt = w0 + w1
                            if (not real) and ablate == "noexp":
                                p16 = pconst
                            else:
                                p16 = ev.tile(
                                    [128, 1024], F16, name=f"p_{h}_{j}_{m}", tag="p", bufs=4)
                                if masked:
                                    praw = ev.tile(
                                        [128, 1024], F16, name=f"pr_{h}_{j}_{m}", tag="praw", bufs=3)
                                    nc.scalar.activation(
                                        praw[:, 0:wt], st[:, 0:wt], AF.Exp, scale=SCALE)
                                    nc.vector.tensor_mul(
                                        p16[:, 0:wt], praw[:, 0:wt],
                                        mask_t[:, packoff:packoff + wt]
                                    )
                                else:
                                    nc.scalar.activation(
                                        p16[:, 0:wt], st[:, 0:wt], AF.Exp, scale=SCALE)
                            if pending is not None:
                                pot, ph, pkb0, pkb1, pp16, pw0, pw1, pnkb = pending
                                for i, (kb, w) in enumerate(((pkb0, pw0), (pkb1, pw1))):
                                    nc.tensor.matmul(
                                        pot[0:65, 512 - w:512], v_t[kb][:, ph, :],
                                        pp16[:, i * pw0:i * pw0 + w],
                                        start=(kb == 0), stop=(kb == pnkb - 1),
                                    )
                            pending = (ot, h, kb0, kb1, p16, w0, w1, nkb)
                            if qk_queues:
                                for hp2 in (1, 2, 3):
                                    if qk_queues.get(hp2):
                                        qk_queues[hp2].pop(0)()
                                        break
                        norm_q.append((ot, hp, off, j, h))
                        if len(norm_q) >= 2:
                            e = norm_q.pop(0)
                            do_evict(*e, head_state[e[4]])
                    # head boundary: flush deferred AV, drain evicts,
                    # then batched bcast+muls
                    if pending is not None:
                        pot, ph, pkb0, pkb1, pp16, pw0, pw1, pnkb = pending
                        for i, (kb, w) in enumerate(((pkb0, pw0), (pkb1, pw1))):
                            nc.tensor.matmul(
                                pot[0:65, 512 - w:512], v_t[kb][:, ph, :],
                                pp16[:, i * pw0:i * pw0 + w],
                                start=(kb == 0), stop=(kb == pnkb - 1),
                            )
                        pending = None
                    for e in norm_q:
                        do_evict(*e, head_state[e[4]])
                    norm_q = []
                    flush_head(hp, off, h, head_state[h])
                    del head_state[h]
                if pending is not None:
                    pot, ph, pkb0, pkb1, pp16, pw0, pw1, pnkb = pending
                    for i, (kb, w) in enumerate(((pkb0, pw0), (pkb1, pw1))):
                        nc.tensor.matmul(
                            pot[0:65, 512 - w:512], v_t[kb][:, ph, :],
                            pp16[:, i * pw0:i * pw0 + w],
                            start=(kb == 0), stop=(kb == pnkb - 1),
                        )
                    pending = None

            def phase3():
                # ---- phase 3: output projection (partial) ----
                for cc in range(C // 128):
                    ys = ev.tile([128, T], F32, name=f"ys_{cc}", tag="ys", bufs=2)
                    for qs in range(NTS):
                        py = ps.tile([128, 512], F32, name=f"py_{cc}_{qs}", tag="st", bufs=3)
                        for hp in range(NHP):
                            nc.tensor.matmul(
                                py[:],
                                wo_t[hp][:, cc * 128:(cc + 1) * 128],
                                ot_t[hp][:, qs * 512:(qs + 1) * 512],
                                start=(hp == 0), stop=(hp == NHP - 1),
                            )
                        nc.vector.tensor_copy(ys[:, qs * 512:(qs + 1) * 512], py[:])
                    nc.sync.dma_start(yt[cc * 128:(cc + 1) * 128, :], ys[:])

            if loop_iters > 1:
                if tuple(loop_phases) != (1, 2, 3) or ablate is not None:
                    body(real=True)  # populate intermediates once
                with tc.For_i(0, loop_iters, 1):
                    body(tuple(loop_phases))
            else:
                body()

    nc.compile()
    _cache[key] = nc
    return nc


def _make_masks():
    # packed diagonal masks: pack0 = [tril 512 | tril 384], pack1 = [tril 256 | tril 128]
    kk = np.arange(128)[:, None]
    m = np.zeros((128, 1280), dtype=np.float16)
    off = 0
    for w in (512, 384, 256, 128):
        qq = np.arange(w)[None, :]
        m[:, off:off + w] = (kk <= qq).astype(np.float16)
        off += w
    return m


def kernel(x, W_qkv, b_qkv, W_out, b_out):
    x = np.asarray(x, dtype=np.float32)
    W_qkv = np.asarray(W_qkv, dtype=np.float32)
    b_qkv = np.asarray(b_qkv, dtype=np.float32)
    W_out = np.asarray(W_out, dtype=np.float32)
    b_out = np.asarray(b_out, dtype=np.float32)

    nc = _build()
    masks = _make_masks()

    in_maps = []
    for core in range(8):
        b, g = core // 2, core % 2
        sl = slice(g * HPC, (g + 1) * HPC)
        bq_c = b_qkv[0 * C:1 * C][sl]
        bk_c = b_qkv[1 * C:2 * C][sl]
        bv_c = b_qkv[2 * C:3 * C][sl]
        in_maps.append(dict(
            xt=np.ascontiguousarray(x[b].T).astype(np.float16),
            wq=W_qkv[:, 0 * C:1 * C][:, sl].astype(np.float16),
            wk=W_qkv[:, 1 * C:2 * C][:, sl].astype(np.float16),
            wv=W_qkv[:, 2 * C:3 * C][:, sl].astype(np.float16),
            wo=W_out[sl, :].astype(np.float16),
            bqk=np.concatenate(
                [bq_c.reshape(NHP, 128).T, bk_c.reshape(NHP, 128).T], axis=1
            ).astype(np.float32),
            bv=np.tile(bv_c[None, :], (128, 1)).astype(np.float16),
            masks=masks,
        ))

    res = bass_utils.run_bass_kernel_spmd(nc, in_maps, core_ids=list(range(8)))
    out = np.zeros((B, T, C), dtype=np.float32)
    for core in range(8):
        b = core // 2
        out[b] += res.results[core]["yt"].T
    out += b_out[None, None, :]
    return out
